# revision 7
# baseline (speedup 1.0000x reference)
"""Bahdanau attention Trainium2 kernel (transposed-hp design).

B=32, T=1, S=4096, H=1024. Data-parallel over batch across 8 NeuronCores
(4 batches/core). Per core, a single-pass streaming kernel built around a
transposed h_proj layout hp^T[o, s] so that:

  - the q_proj bias-add fuses into ScalarE's tanh as a per-partition bias
    (VectorE drops out of the inner loop entirely)
  - the v-dot score reduction becomes cheap DoubleRow fp8 matmuls
    (v as a [K,1] stationary)
  - TensorE runs dense (h_proj fp8-DR, score MMs, ctx MMs, final MMs)
    and stays HAM-warm

Per chunk of 512 encoder positions: enc streams HBM->bf16 natural (SWDGE
cast) -> xbar transpose (bf16) -> fp8 copy on VectorE. h_proj^T[o,s]
accumulates Wh^T(fp8,stationary) x encT8(fp8,moving) with DoubleRow;
ScalarE applies tanh(hp + q_proj[o]) writing fp8 tiles; v^T-dot scores via
fp8-DR MMs into a [1,512] PSUM row; exp (scale=1/16 compensates v8's x16)
accumulates the softmax denominator; tiny PE transposes give attn columns
for the bf16 ctx matmuls which trail one chunk behind.

The final out = tanh(Wout @ [ctx; q]) uses a hi/lo split-bf16 Wout and
split cat (3 accumulation chains), giving ~fp32 accuracy (~3e-3 rel).

softmax is computed without max-subtraction: scores stay O(1) for this
data; exp accumulates in fp32.

src_lengths is (faithfully to the reference) unused.
"""
import numpy as np
from contextlib import ExitStack

import concourse.bass as bass
import concourse.tile as tile
from concourse import bacc, mybir, masks
from concourse import bass_isa
from concourse import bass_utils

F32 = mybir.dt.float32
BF16 = mybir.dt.bfloat16
FP8 = mybir.dt.float8e4
Tanh = mybir.ActivationFunctionType.Tanh
Exp = mybir.ActivationFunctionType.Exp
Copy = mybir.ActivationFunctionType.Copy
DR = mybir.MatmulPerfMode.DoubleRow

B, T, S, H = 32, 1, 4096, 1024
NCORES = 8
BL = B // NCORES       # batches per core
NS = S // 128          # s-tiles per batch
CH = 4                 # s-tiles per enc chunk (512 positions)
NCH = NS // CH         # chunks per batch
NHB = H // 128         # h (contraction) blocks
NOB = H // 128         # o (output) blocks
NKB = 2 * H // 128     # k blocks of cat=[ctx;query]
PREFETCH = 2           # chunks of lookahead on the enc stream


def _build_program():
    nc = bacc.Bacc("TRN2", target_bir_lowering=False, debug=False)

    q_d = nc.dram_tensor("query", (BL, T, H), F32, kind="ExternalInput").ap()
    enc_d = nc.dram_tensor("encoder_outputs", (BL, S, H), F32,
                           kind="ExternalInput").ap()
    ws_d = nc.dram_tensor("Ws_w", (H, H), F32, kind="ExternalInput").ap()
    wh_d = nc.dram_tensor("Wh_w", (H, H), F32, kind="ExternalInput").ap()
    v_d = nc.dram_tensor("v_w", (1, H), F32, kind="ExternalInput").ap()
    wout_d = nc.dram_tensor("Wout_w", (H, 2 * H), F32, kind="ExternalInput").ap()
    out_d = nc.dram_tensor("out", (BL, T, H), F32, kind="ExternalOutput").ap()

    with tile.TileContext(nc) as tc, ExitStack() as ctx:
        # ---------------- pools ----------------
        wt_pool = ctx.enter_context(tc.tile_pool(name="wt", bufs=1))
        wstage_pool = ctx.enter_context(tc.tile_pool(name="wstage", bufs=1))
        wnat_pool = ctx.enter_context(tc.tile_pool(name="wnat", bufs=4))
        wof_pool = ctx.enter_context(tc.tile_pool(name="wof", bufs=1))
        wohl_pool = ctx.enter_context(tc.tile_pool(name="wohl", bufs=2))
        encN_pool = ctx.enter_context(tc.tile_pool(name="encN", bufs=3))
        encT_pool = ctx.enter_context(tc.tile_pool(name="encT", bufs=3))
        encT8_pool = ctx.enter_context(tc.tile_pool(name="encT8", bufs=3))
        t8_pool = ctx.enter_context(tc.tile_pool(name="t8", bufs=2))
        att_pool = ctx.enter_context(tc.tile_pool(name="att", bufs=2))
        small_pool = ctx.enter_context(tc.tile_pool(name="small", bufs=1))

        hp_psum = ctx.enter_context(tc.tile_pool(name="hp_ps", bufs=3, space="PSUM"))
        sc_psum = ctx.enter_context(tc.tile_pool(name="sc_ps", bufs=2, space="PSUM"))
        ctx_psum = ctx.enter_context(tc.tile_pool(name="ctx_ps", bufs=1, space="PSUM"))
        atp_psum = ctx.enter_context(tc.tile_pool(name="atp_ps", bufs=1, space="PSUM"))

        # ---------------- constants ----------------
        id1 = small_pool.tile([1, 1], F32)
        masks.make_identity(nc, id1[:])
        id4 = small_pool.tile([4, 4], F32)
        masks.make_identity(nc, id4[:])
        id8 = small_pool.tile([8, 8], F32)
        masks.make_identity(nc, id8[:])

        # ---------------- Wh path: nat casts -> xbar transpose -> fp8 ------
        whT = wstage_pool.tile([128, NHB, H], BF16, tag="whT")
        whT8 = wt_pool.tile([128, NHB, H], FP8, tag="whT8")
        wh_nat = []
        for j in range(NHB):
            wN = wnat_pool.tile([128, H], BF16, tag="wnat")
            nc.gpsimd.dma_start(wN[:], wh_d[j * 128:(j + 1) * 128, :])
            wh_nat.append(wN)
        for j in range(NHB):
            nc.sync.dma_start(whT[:, :, j * 128:(j + 1) * 128], wh_nat[j][:],
                              transpose=True)
            nc.vector.tensor_copy(whT8[:, :, j * 128:(j + 1) * 128],
                                  whT[:, :, j * 128:(j + 1) * 128])

        # ---------------- encoder chunk loader ----------------
        chunk_tiles = {}

        def chunk_dma(b, c):
            if (b, c) in chunk_tiles:
                return chunk_tiles[(b, c)]
            encN = encN_pool.tile([128, CH, H], BF16, tag="encN")
            src = enc_d[b, c * CH * 128:(c + 1) * CH * 128, :]
            nc.gpsimd.dma_start(encN[:], src.rearrange("(t p) h -> p t h", p=128))
            encT = encT_pool.tile([128, CH * NHB, 128], BF16, tag="encT")
            nc.sync.dma_start(encT[:], encN[:], transpose=True)
            encT8 = encT8_pool.tile([128, CH, NHB, 128], FP8, tag="encT8")
            nc.vector.tensor_copy(
                encT8[:].rearrange("p t k i -> p (t k) i"), encT[:])
            chunk_tiles[(b, c)] = (encN, encT8)
            return chunk_tiles[(b, c)]

        for c in range(PREFETCH + 1):
            chunk_dma(0, c)

        # ---------------- q / v / Ws path ----------------
        q_sb = small_pool.tile([BL, H], F32)
        nc.gpsimd.dma_start(q_sb[:], q_d[0:BL, 0, :])
        v_sb = small_pool.tile([NHB, 128], F32)
        for j in range(NHB):
            nc.gpsimd.dma_start(v_sb[j:j + 1, :], v_d[0:1, j * 128:(j + 1) * 128])

        wsT = wt_pool.tile([128, NHB, H], BF16, tag="wbig")
        ws_nat = []
        for j in range(NHB):
            wN = wnat_pool.tile([128, H], BF16, tag="wnat")
            nc.gpsimd.dma_start(wN[:], ws_d[j * 128:(j + 1) * 128, :])
            ws_nat.append(wN)
        for j in range(NHB):
            nc.sync.dma_start(wsT[:, :, j * 128:(j + 1) * 128], ws_nat[j][:],
                              transpose=True)

        # q transposed: [h, b] f32 -> bf16 for the q_proj matmuls, and the
        # hi/lo split halves of cat's query blocks
        cat_hi = small_pool.tile([128, NKB, BL], BF16)
        cat_lo = small_pool.tile([128, NKB, BL], BF16)
        qT = small_pool.tile([128, NHB, BL], BF16)
        qt_ps = atp_psum.tile([128, NHB * BL], F32, tag="atp")
        for j in range(NHB):
            nc.tensor.transpose(qt_ps[:, j * BL:(j + 1) * BL],
                                q_sb[0:BL, j * 128:(j + 1) * 128], id4[:])
        nc.scalar.copy(qT[:], qt_ps[:])
        nc.scalar.copy(cat_hi[:, NHB:NKB, :], qt_ps[:])
        nc.vector.tensor_sub(cat_lo[:, NHB:NKB, :], qt_ps[:],
                             cat_hi[:, NHB:NKB, :])

        # v transposed to [o%128, ob] and scaled x16 into fp8 (padded stride
        # 16 so DoubleRow pair-slices have a legal step)
        v8 = small_pool.tile([128, NHB, 16], FP8)
        vt_ps = atp_psum.tile([128, NHB], F32, tag="atp")
        nc.tensor.transpose(vt_ps[:], v_sb[:], id8[:])
        nc.scalar.activation(v8[:, :, 0:1], vt_ps[:].unsqueeze(-1), Copy,
                             scale=16.0)

        # q_proj^T[o, b] via PE, then to SBUF f32 as tanh's per-partition bias
        qpT = small_pool.tile([128, NOB, BL], F32)
        qp_ps = atp_psum.tile([128, NOB * BL], F32, tag="atp")
        for ob in range(NOB):
            for hb in range(NHB):
                nc.tensor.matmul(qp_ps[:, ob * BL:(ob + 1) * BL],
                                 wsT[:, hb, ob * 128:(ob + 1) * 128],
                                 qT[:, hb, :],
                                 start=(hb == 0), stop=(hb == NHB - 1))
        nc.scalar.copy(qpT[:], qp_ps[:].rearrange("p (o b) -> p o b", b=BL))

        # ---------------- Wout hi/lo prep (deferred, off critical path) ----
        woutT_hi = wt_pool.tile([128, NKB, H], BF16, tag="wbig")
        woutT_lo = wt_pool.tile([128, NKB, H], BF16, tag="wlo")
        for j in range(NHB):
            with tc.tile_wait_until(0.06 + j * 0.03):
                wof = wof_pool.tile([128, 2 * H], F32, tag="wof")
                nc.gpsimd.dma_start(wof[:], wout_d[j * 128:(j + 1) * 128, :])
                w_hi = wohl_pool.tile([128, 2 * H], BF16, tag="wohl")
                nc.gpsimd.dma_start(w_hi[:], wof[:])
                w_lo = wohl_pool.tile([128, 2 * H], BF16, tag="wohl")
                nc.vector.tensor_sub(w_lo[:], wof[:], w_hi[:])
            with tc.tile_wait_until(0.09 + j * 0.03):
                nc.sync.dma_start(woutT_hi[:, :, j * 128:(j + 1) * 128],
                                  w_hi[:], transpose=True)
                nc.sync.dma_start(woutT_lo[:, :, j * 128:(j + 1) * 128],
                                  w_lo[:], transpose=True)

        # ---------------- main loop ----------------
        ctx_sb = small_pool.tile([BL, H], F32)

        for b in range(BL):
            attnT = att_pool.tile([128, NS], BF16, tag="attnT")
            dsum = att_pool.tile([1, NCH], F32, tag="dsum")
            ctx_ps = ctx_psum.tile([1, H], F32, tag="ctx")
            enc_chunks = [None] * NCH

            def emit_ctx(c):
                encN = enc_chunks[c]
                for t in range(CH):
                    st = c * CH + t
                    for half in range(2):
                        nc.tensor.matmul(
                            ctx_ps[0:1, half * 512:(half + 1) * 512],
                            attnT[:, st:st + 1],
                            encN[:, t, half * 512:(half + 1) * 512],
                            start=(st == 0), stop=(st == NS - 1))

            for c in range(NCH):
                encN, encT8 = chunk_dma(b, c)
                enc_chunks[c] = encN
                g = b * NCH + c + PREFETCH
                if g < BL * NCH:
                    chunk_dma(g // NCH, g % NCH)

                t8 = t8_pool.tile([128, NOB, 512], FP8, tag="t8")
                sc_ps = sc_psum.tile([1, 512], F32, tag="sc")
                # moving operand for h_proj: [Ki, Ko-pair, (t, i)] 4D view
                enc_mv = encT8[:].rearrange("p t k i -> p k t i")
                for ob in range(NOB):
                    hp = hp_psum.tile([128, 512], F32, tag="hp")
                    for kp in range(NHB // 2):
                        nc.tensor.matmul(
                            hp[:],
                            whT8[:, 2 * kp:2 * kp + 2, ob * 128:(ob + 1) * 128],
                            enc_mv[:, 2 * kp:2 * kp + 2, :, :],
                            start=(kp == 0), stop=(kp == NHB // 2 - 1),
                            perf_mode=DR)
                    nc.scalar.activation(t8[:, ob, :], hp[:], Tanh,
                                         bias=qpT[:, ob, b:b + 1])
                    if ob % 2 == 1:
                        nc.tensor.matmul(
                            sc_ps[0:1, :],
                            v8[:, ob - 1:ob + 1, 0:1],
                            t8[:, ob - 1:ob + 1, :],
                            start=(ob == 1), stop=(ob == NOB - 1),
                            perf_mode=DR)
                # exp (scale compensates v8's x16); accumulate denominator
                attnU = att_pool.tile([1, 512], F32, tag="attnU")
                nc.scalar.activation(attnU[0:1, :],
                                     sc_ps[0:1, :], Exp, scale=1.0 / 16.0,
                                     accum_out=dsum[0:1, c:c + 1])
                # attn row -> columns via tiny PE transposes
                atp = atp_psum.tile([128, CH], F32, tag="atp")
                for t in range(CH):
                    nc.tensor.transpose(
                        atp[:, t:t + 1],
                        attnU[0:1, t * 128:(t + 1) * 128],
                        id1[:])
                nc.scalar.copy(attnT[:, c * CH:(c + 1) * CH], atp[:])
                if c >= 1:
                    emit_ctx(c - 1)
            emit_ctx(NCH - 1)

            # denominator & normalize
            dtot = att_pool.tile([1, 1], F32, tag="dtot")
            nc.vector.reduce_sum(dtot[:], dsum[:], axis=mybir.AxisListType.X)
            inv_d = att_pool.tile([1, 1], F32, tag="invd")
            nc.vector.reciprocal(inv_d[:], dtot[:])
            ctx_row = wstage_pool.tile([1, H], F32, tag="ctxrow")
            nc.scalar.activation(ctx_row[:], ctx_ps[:], Copy,
                                 scale=inv_d[0:1, 0:1])
            nc.sync.dma_start(ctx_sb[b:b + 1, :], ctx_row[:])

        # ---------------- finale ----------------
        ct_ps = atp_psum.tile([128, NHB * BL], F32, tag="atp")
        for j in range(NHB):
            nc.tensor.transpose(ct_ps[:, j * BL:(j + 1) * BL],
                                ctx_sb[0:BL, j * 128:(j + 1) * 128], id4[:])
        nc.scalar.copy(cat_hi[:, 0:NHB, :], ct_ps[:])
        nc.vector.tensor_sub(cat_lo[:, 0:NHB, :], ct_ps[:], cat_hi[:, 0:NHB, :])

        out_sb = small_pool.tile([BL, H], F32)
        chains = [(cat_hi, woutT_hi), (cat_hi, woutT_lo), (cat_lo, woutT_hi)]
        for half in range(2):
            out_ps = sc_psum.tile([BL, 512], F32, tag="sc")
            n = 0
            for ca, wb in chains:
                for kb in range(NKB):
                    nc.tensor.matmul(out_ps[:], ca[:, kb, :],
                                     wb[:, kb, half * 512:(half + 1) * 512],
                                     start=(n == 0),
                                     stop=(n == 3 * NKB - 1))
                    n += 1
            nc.scalar.activation(out_sb[:, half * 512:(half + 1) * 512],
                                 out_ps[:], Tanh)
        nc.sync.dma_start(out_d[0:BL, 0, :], out_sb[:])

    nc.compile()
    return nc


_program = None


def get_program():
    global _program
    if _program is None:
        _program = _build_program()
    return _program


def run_sharded(inputs, trace=False, **kw):
    nc = get_program()
    in_maps = []
    for i in range(NCORES):
        sl = slice(i * BL, (i + 1) * BL)
        in_maps.append({
            "query": np.ascontiguousarray(inputs["query"][sl], dtype=np.float32),
            "encoder_outputs": np.ascontiguousarray(
                inputs["encoder_outputs"][sl], dtype=np.float32),
            "Ws_w": np.asarray(inputs["Ws_w"], dtype=np.float32),
            "Wh_w": np.asarray(inputs["Wh_w"], dtype=np.float32),
            "v_w": np.asarray(inputs["v_w"], dtype=np.float32),
            "Wout_w": np.asarray(inputs["Wout_w"], dtype=np.float32),
        })
    res = bass_utils.run_bass_kernel_spmd(
        nc, in_maps, core_ids=list(range(NCORES)), trace=trace, **kw)
    out = np.concatenate(
        [np.asarray(res.results[i]["out"], dtype=np.float32).reshape(BL, T, H)
         for i in range(NCORES)], axis=0)
    return out, res


def kernel(**inputs):
    out, _ = run_sharded(inputs)
    return out


# revision 10
# speedup vs baseline: 1.1456x; 1.1456x over previous
"""Bahdanau attention Trainium2 kernel (transposed-hp design).

B=32, T=1, S=4096, H=1024. Data-parallel over batch across 8 NeuronCores
(4 batches/core). Per core, a single-pass streaming kernel built around a
transposed h_proj layout hp^T[o, s] so that:

  - the q_proj bias-add fuses into ScalarE's tanh as a per-partition bias
    (VectorE drops out of the inner loop entirely)
  - the v-dot score reduction becomes cheap DoubleRow fp8 matmuls
    (v as a [K,1] stationary)
  - TensorE runs dense (h_proj fp8-DR, score MMs, ctx MMs, final MMs)
    and stays HAM-warm

Per chunk of 512 encoder positions: enc streams HBM->bf16 natural (SWDGE
cast) -> xbar transpose (bf16) -> fp8 copy on VectorE. h_proj^T[o,s]
accumulates Wh^T(fp8,stationary) x encT8(fp8,moving) with DoubleRow;
ScalarE applies tanh(hp + q_proj[o]) writing fp8 tiles; v^T-dot scores via
fp8-DR MMs into a [1,512] PSUM row; exp (scale=1/16 compensates v8's x16)
accumulates the softmax denominator; tiny PE transposes give attn columns
for the bf16 ctx matmuls.

Each chunk's softmax/ctx tail is software-pipelined into the NEXT chunk's
emission at points where its cross-engine dependencies are already
settled, so the in-order PE queue never waits on ScalarE: score pairs
trail their tanh by 2 ob-groups, the chunk's last score pair + exp +
attn-transposes + ctx matmuls trail by one chunk.

The final out = tanh(Wout @ [ctx; q]) uses a hi/lo split-bf16 Wout and
split cat (3 accumulation chains), giving ~fp32 accuracy (~4e-3 rel).

softmax is computed without max-subtraction: scores stay O(1) for this
data; exp accumulates in fp32.

src_lengths is (faithfully to the reference) unused.
"""
import numpy as np
from contextlib import ExitStack

import concourse.bass as bass
import concourse.tile as tile
from concourse import bacc, mybir, masks
from concourse import bass_isa
from concourse import bass_utils

F32 = mybir.dt.float32
BF16 = mybir.dt.bfloat16
FP8 = mybir.dt.float8e4
Tanh = mybir.ActivationFunctionType.Tanh
Exp = mybir.ActivationFunctionType.Exp
Copy = mybir.ActivationFunctionType.Copy
DR = mybir.MatmulPerfMode.DoubleRow

B, T, S, H = 32, 1, 4096, 1024
NCORES = 8
BL = B // NCORES       # batches per core
NS = S // 128          # s-tiles per batch
CH = 4                 # s-tiles per enc chunk (512 positions)
NCH = NS // CH         # chunks per batch
NHB = H // 128         # h (contraction) blocks
NOB = H // 128         # o (output) blocks
NKB = 2 * H // 128     # k blocks of cat=[ctx;query]
PREFETCH = 2           # chunks of lookahead on the enc stream


def _build_program():
    nc = bacc.Bacc("TRN2", target_bir_lowering=False, debug=False)

    q_d = nc.dram_tensor("query", (BL, T, H), F32, kind="ExternalInput").ap()
    enc_d = nc.dram_tensor("encoder_outputs", (BL, S, H), F32,
                           kind="ExternalInput").ap()
    ws_d = nc.dram_tensor("Ws_w", (H, H), F32, kind="ExternalInput").ap()
    wh_d = nc.dram_tensor("Wh_w", (H, H), F32, kind="ExternalInput").ap()
    v_d = nc.dram_tensor("v_w", (1, H), F32, kind="ExternalInput").ap()
    wout_d = nc.dram_tensor("Wout_w", (H, 2 * H), F32, kind="ExternalInput").ap()
    out_d = nc.dram_tensor("out", (BL, T, H), F32, kind="ExternalOutput").ap()

    with tile.TileContext(nc) as tc, ExitStack() as ctx:
        # ---------------- pools ----------------
        wt_pool = ctx.enter_context(tc.tile_pool(name="wt", bufs=1))
        wstage_pool = ctx.enter_context(tc.tile_pool(name="wstage", bufs=1))
        wnat_pool = ctx.enter_context(tc.tile_pool(name="wnat", bufs=3))
        wof_pool = ctx.enter_context(tc.tile_pool(name="wof", bufs=1))
        wohl_pool = ctx.enter_context(tc.tile_pool(name="wohl", bufs=2))
        encN_pool = ctx.enter_context(tc.tile_pool(name="encN", bufs=4))
        encT_pool = ctx.enter_context(tc.tile_pool(name="encT", bufs=3))
        encT8_pool = ctx.enter_context(tc.tile_pool(name="encT8", bufs=3))
        t8_pool = ctx.enter_context(tc.tile_pool(name="t8", bufs=2))
        att_pool = ctx.enter_context(tc.tile_pool(name="att", bufs=2))
        small_pool = ctx.enter_context(tc.tile_pool(name="small", bufs=1))

        hp_psum = ctx.enter_context(tc.tile_pool(name="hp_ps", bufs=3, space="PSUM"))
        sc_psum = ctx.enter_context(tc.tile_pool(name="sc_ps", bufs=2, space="PSUM"))
        ctx_psum = ctx.enter_context(tc.tile_pool(name="ctx_ps", bufs=1, space="PSUM"))
        atp_psum = ctx.enter_context(tc.tile_pool(name="atp_ps", bufs=1, space="PSUM"))

        # ---------------- constants ----------------
        id1 = small_pool.tile([1, 1], F32)
        masks.make_identity(nc, id1[:])
        id4 = small_pool.tile([4, 4], F32)
        masks.make_identity(nc, id4[:])
        id8 = small_pool.tile([8, 8], F32)
        masks.make_identity(nc, id8[:])

        # ---------------- encoder chunk loader ----------------
        chunk_tiles = {}

        def chunk_dma(g):
            if g in chunk_tiles:
                return chunk_tiles[g]
            b, c = g // NCH, g % NCH
            encN = encN_pool.tile([128, CH, H], BF16, tag="encN")
            src = enc_d[b, c * CH * 128:(c + 1) * CH * 128, :]
            nc.gpsimd.dma_start(encN[:], src.rearrange("(t p) h -> p t h", p=128))
            encT = encT_pool.tile([128, CH * NHB, 128], BF16, tag="encT")
            nc.sync.dma_start(encT[:], encN[:], transpose=True)
            encT8 = encT8_pool.tile([128, CH, NHB, 128], FP8, tag="encT8")
            nc.vector.tensor_copy(
                encT8[:].rearrange("p t k i -> p (t k) i"), encT[:])
            chunk_tiles[g] = (encN, encT8)
            return chunk_tiles[g]

        # first enc chunks go out before anything else
        chunk_dma(0)
        chunk_dma(1)

        # ---------------- Wh path: nat casts -> xbar transpose -> fp8 ------
        whT = wstage_pool.tile([128, NHB, H], BF16, tag="whT")
        whT8 = wt_pool.tile([128, NHB, H], FP8, tag="whT8")
        wh_nat = []
        for j in range(NHB):
            wN = wnat_pool.tile([128, H], BF16, tag="wnat")
            nc.gpsimd.dma_start(wN[:], wh_d[j * 128:(j + 1) * 128, :])
            wh_nat.append(wN)
        for j in range(NHB):
            nc.sync.dma_start(whT[:, :, j * 128:(j + 1) * 128], wh_nat[j][:],
                              transpose=True)
            nc.vector.tensor_copy(whT8[:, :, j * 128:(j + 1) * 128],
                                  whT[:, :, j * 128:(j + 1) * 128])

        chunk_dma(2)

        # ---------------- q / v / Ws path ----------------
        q_sb = small_pool.tile([BL, H], F32)
        nc.gpsimd.dma_start(q_sb[:], q_d[0:BL, 0, :])
        v_sb = small_pool.tile([NHB, 128], F32)
        for j in range(NHB):
            nc.gpsimd.dma_start(v_sb[j:j + 1, :], v_d[0:1, j * 128:(j + 1) * 128])

        wsT = wt_pool.tile([128, NHB, H], BF16, tag="wbig")
        ws_nat = []
        for j in range(NHB):
            wN = wnat_pool.tile([128, H], BF16, tag="wnat")
            nc.gpsimd.dma_start(wN[:], ws_d[j * 128:(j + 1) * 128, :])
            ws_nat.append(wN)
        for j in range(NHB):
            nc.sync.dma_start(wsT[:, :, j * 128:(j + 1) * 128], ws_nat[j][:],
                              transpose=True)

        # q transposed: [h, b] f32 -> bf16 for the q_proj matmuls, and the
        # hi/lo split halves of cat's query blocks
        cat_hi = small_pool.tile([128, NKB, BL], BF16)
        cat_lo = small_pool.tile([128, NKB, BL], BF16)
        qT = small_pool.tile([128, NHB, BL], BF16)
        qt_ps = atp_psum.tile([128, NHB * BL], F32, tag="atp")
        for j in range(NHB):
            nc.tensor.transpose(qt_ps[:, j * BL:(j + 1) * BL],
                                q_sb[0:BL, j * 128:(j + 1) * 128], id4[:])
        nc.scalar.copy(qT[:], qt_ps[:])
        nc.scalar.copy(cat_hi[:, NHB:NKB, :], qt_ps[:])
        nc.vector.tensor_sub(cat_lo[:, NHB:NKB, :], qt_ps[:],
                             cat_hi[:, NHB:NKB, :])

        # v transposed to [o%128, ob] and scaled x16 into fp8 (padded stride
        # 16 so DoubleRow pair-slices have a legal step)
        v8 = small_pool.tile([128, NHB, 16], FP8)
        vt_ps = atp_psum.tile([128, NHB], F32, tag="atp")
        nc.tensor.transpose(vt_ps[:], v_sb[:], id8[:])
        nc.scalar.activation(v8[:, :, 0:1], vt_ps[:].unsqueeze(-1), Copy,
                             scale=16.0)

        # q_proj^T[o, b] via PE, then to SBUF f32 as tanh's per-partition bias
        qpT = small_pool.tile([128, NOB, BL], F32)
        qp_ps = atp_psum.tile([128, NOB * BL], F32, tag="atp")
        for ob in range(NOB):
            for hb in range(NHB):
                nc.tensor.matmul(qp_ps[:, ob * BL:(ob + 1) * BL],
                                 wsT[:, hb, ob * 128:(ob + 1) * 128],
                                 qT[:, hb, :],
                                 start=(hb == 0), stop=(hb == NHB - 1))
        nc.scalar.copy(qpT[:], qp_ps[:].rearrange("p (o b) -> p o b", b=BL))

        # ---------------- Wout hi/lo prep (deferred, off critical path) ----
        woutT_hi = wt_pool.tile([128, NKB, H], BF16, tag="wbig")
        woutT_lo = wt_pool.tile([128, NKB, H], BF16, tag="wlo")
        for j in range(NHB):
            with tc.tile_wait_until(0.05 + j * 0.045):
                wof = wof_pool.tile([128, 2 * H], F32, tag="wof")
                nc.gpsimd.dma_start(wof[:], wout_d[j * 128:(j + 1) * 128, :])
                w_hi = wohl_pool.tile([128, 2 * H], BF16, tag="wohl")
                nc.gpsimd.dma_start(w_hi[:], wof[:])
                w_lo = wohl_pool.tile([128, 2 * H], BF16, tag="wohl")
                nc.vector.tensor_sub(w_lo[:], wof[:], w_hi[:])
            with tc.tile_wait_until(0.08 + j * 0.045):
                nc.sync.dma_start(woutT_hi[:, :, j * 128:(j + 1) * 128],
                                  w_hi[:], transpose=True)
                nc.sync.dma_start(woutT_lo[:, :, j * 128:(j + 1) * 128],
                                  w_lo[:], transpose=True)

        # ---------------- main loop (software-pipelined chunk tails) -------
        ctx_sb = small_pool.tile([BL, H], F32)
        NG = BL * NCH
        # per-chunk state carried into the next block
        st_t8 = {}
        st_sc = {}
        st_att = {}   # batch -> (attnT, dsum, ctx_ps)

        def hp_group(g, ob, encT8, t8, bi):
            hp = hp_psum.tile([128, 512], F32, tag="hp")
            enc_mv = encT8[:].rearrange("p t k i -> p k t i")
            for kp in range(NHB // 2):
                nc.tensor.matmul(
                    hp[:],
                    whT8[:, 2 * kp:2 * kp + 2, ob * 128:(ob + 1) * 128],
                    enc_mv[:, 2 * kp:2 * kp + 2, :, :],
                    start=(kp == 0), stop=(kp == NHB // 2 - 1),
                    perf_mode=DR)
            nc.scalar.activation(t8[:, ob, :], hp[:], Tanh,
                                 bias=qpT[:, ob, bi:bi + 1])

        def scores_mm(g, p, t8, sc_ps):
            nc.tensor.matmul(sc_ps[0:1, :],
                             v8[:, 2 * p:2 * p + 2, 0:1],
                             t8[:, 2 * p:2 * p + 2, :],
                             start=(p == 0), stop=(p == NOB // 2 - 1),
                             perf_mode=DR)

        def tail_a(g):
            # final score pair + exp for chunk g (tanh(g,6..7) long done)
            pb, pc = g // NCH, g % NCH
            scores_mm(g, NOB // 2 - 1, st_t8.pop(g), st_sc[g])
            attnT, dsum, _ = st_att[pb]
            attnU = att_pool.tile([1, 512], F32, tag="attnU")
            nc.scalar.activation(attnU[0:1, :], st_sc.pop(g)[0:1, :], Exp,
                                 scale=1.0 / 16.0,
                                 accum_out=dsum[0:1, pc:pc + 1])
            return attnU

        def tail_b(g, attnU):
            # attn row -> columns (PE) and bf16 copy (ACT)
            pb, pc = g // NCH, g % NCH
            attnT, _, _ = st_att[pb]
            atp = atp_psum.tile([128, CH], F32, tag="atp")
            for t in range(CH):
                nc.tensor.transpose(atp[:, t:t + 1],
                                    attnU[0:1, t * 128:(t + 1) * 128], id1[:])
            nc.scalar.copy(attnT[:, pc * CH:(pc + 1) * CH], atp[:])

        def tail_c(g):
            # ctx matmuls for chunk g
            pb, pc = g // NCH, g % NCH
            attnT, _, ctx_ps = st_att[pb]
            encN = chunk_tiles[g][0]
            for t in range(CH):
                st = pc * CH + t
                for half in range(2):
                    nc.tensor.matmul(
                        ctx_ps[0:1, half * 512:(half + 1) * 512],
                        attnT[:, st:st + 1],
                        encN[:, t, half * 512:(half + 1) * 512],
                        start=(st == 0), stop=(st == NS - 1))

        def tail_d(g):
            # batch finish: denominator, 1/d, normalized ctx row
            pb = g // NCH
            _, dsum, ctx_ps = st_att.pop(pb)
            dtot = att_pool.tile([1, 1], F32, tag="dtot")
            nc.vector.reduce_sum(dtot[:], dsum[:], axis=mybir.AxisListType.X)
            inv_d = att_pool.tile([1, 1], F32, tag="invd")
            nc.vector.reciprocal(inv_d[:], dtot[:])
            ctx_row = wstage_pool.tile([1, H], F32, tag="ctxrow")
            nc.scalar.activation(ctx_row[:], ctx_ps[:], Copy,
                                 scale=inv_d[0:1, 0:1])
            nc.scalar.dma_start(ctx_sb[pb:pb + 1, :], ctx_row[:])

        for g in range(NG):
            b, c = g // NCH, g % NCH
            if c == 0:
                st_att[b] = (
                    att_pool.tile([128, NS], BF16, tag="attnT", name="attnT"),
                    att_pool.tile([1, NCH], F32, tag="dsum", name="dsum"),
                    ctx_psum.tile([1, H], F32, tag="ctx", name="ctx_ps"))
            encN, encT8 = chunk_dma(g)
            if g + PREFETCH < NG:
                chunk_dma(g + PREFETCH)
            t8 = t8_pool.tile([128, NOB, 512], FP8, tag="t8")
            sc_ps = sc_psum.tile([1, 512], F32, tag="sc")

            hp_group(g, 0, encT8, t8, b)
            if g >= 1:
                attnU_prev = tail_a(g - 1)
            hp_group(g, 1, encT8, t8, b)
            hp_group(g, 2, encT8, t8, b)
            if g >= 1:
                tail_b(g - 1, attnU_prev)
            hp_group(g, 3, encT8, t8, b)
            scores_mm(g, 0, t8, sc_ps)
            if g >= 1:
                tail_c(g - 1)
                if (g - 1) % NCH == NCH - 1:
                    tail_d(g - 1)
            hp_group(g, 4, encT8, t8, b)
            hp_group(g, 5, encT8, t8, b)
            scores_mm(g, 1, t8, sc_ps)
            hp_group(g, 6, encT8, t8, b)
            hp_group(g, 7, encT8, t8, b)
            scores_mm(g, 2, t8, sc_ps)
            st_t8[g] = t8
            st_sc[g] = sc_ps

        # epilogue for the last chunk
        attnU_prev = tail_a(NG - 1)
        tail_b(NG - 1, attnU_prev)
        tail_c(NG - 1)
        tail_d(NG - 1)

        # ---------------- finale ----------------
        ct_ps = atp_psum.tile([128, NHB * BL], F32, tag="atp")
        for j in range(NHB):
            nc.tensor.transpose(ct_ps[:, j * BL:(j + 1) * BL],
                                ctx_sb[0:BL, j * 128:(j + 1) * 128], id4[:])
        nc.scalar.copy(cat_hi[:, 0:NHB, :], ct_ps[:])
        nc.vector.tensor_sub(cat_lo[:, 0:NHB, :], ct_ps[:], cat_hi[:, 0:NHB, :])

        out_sb = small_pool.tile([BL, H], F32)
        chains = [(cat_hi, woutT_hi), (cat_hi, woutT_lo), (cat_lo, woutT_hi)]
        for half in range(2):
            out_ps = sc_psum.tile([BL, 512], F32, tag="sc")
            n = 0
            for ca, wb in chains:
                for kb in range(NKB):
                    nc.tensor.matmul(out_ps[:], ca[:, kb, :],
                                     wb[:, kb, half * 512:(half + 1) * 512],
                                     start=(n == 0),
                                     stop=(n == 3 * NKB - 1))
                    n += 1
            nc.scalar.activation(out_sb[:, half * 512:(half + 1) * 512],
                                 out_ps[:], Tanh)
        nc.scalar.dma_start(out_d[0:BL, 0, :], out_sb[:])

    nc.compile()
    return nc


_program = None


def get_program():
    global _program
    if _program is None:
        _program = _build_program()
    return _program


def run_sharded(inputs, trace=False, **kw):
    nc = get_program()
    in_maps = []
    for i in range(NCORES):
        sl = slice(i * BL, (i + 1) * BL)
        in_maps.append({
            "query": np.ascontiguousarray(inputs["query"][sl], dtype=np.float32),
            "encoder_outputs": np.ascontiguousarray(
                inputs["encoder_outputs"][sl], dtype=np.float32),
            "Ws_w": np.asarray(inputs["Ws_w"], dtype=np.float32),
            "Wh_w": np.asarray(inputs["Wh_w"], dtype=np.float32),
            "v_w": np.asarray(inputs["v_w"], dtype=np.float32),
            "Wout_w": np.asarray(inputs["Wout_w"], dtype=np.float32),
        })
    res = bass_utils.run_bass_kernel_spmd(
        nc, in_maps, core_ids=list(range(NCORES)), trace=trace, **kw)
    out = np.concatenate(
        [np.asarray(res.results[i]["out"], dtype=np.float32).reshape(BL, T, H)
         for i in range(NCORES)], axis=0)
    return out, res


def kernel(**inputs):
    out, _ = run_sharded(inputs)
    return out


# revision 14
# speedup vs baseline: 1.1518x; 1.0054x over previous
"""Bahdanau attention Trainium2 kernel (transposed-hp design).

B=32, T=1, S=4096, H=1024. Data-parallel over batch across 8 NeuronCores
(4 batches/core). Per core, a single-pass streaming kernel built around a
transposed h_proj layout hp^T[o, s] so that:

  - the q_proj bias-add fuses into ScalarE's tanh as a per-partition bias
    (VectorE drops out of the inner loop entirely)
  - the v-dot score reduction becomes cheap DoubleRow fp8 matmuls
    (v as a [K,1] stationary)
  - TensorE runs dense (h_proj fp8-DR, score MMs, ctx MMs, final MMs)
    and stays HAM-warm

Per chunk of 512 encoder positions: enc streams HBM->bf16 natural (SWDGE
cast) -> xbar transpose (bf16) -> fp8 copy on VectorE. h_proj^T[o,s]
accumulates Wh^T(fp8,stationary) x encT8(fp8,moving) with DoubleRow;
ScalarE applies tanh(hp + q_proj[o]) writing fp8 tiles; v^T-dot scores via
fp8-DR MMs into a [1,512] PSUM row; exp (scale=1/16 compensates v8's x16)
accumulates the softmax denominator; tiny PE transposes give attn columns
for the bf16 ctx matmuls.

Each chunk's softmax/ctx tail is software-pipelined into the NEXT chunk's
emission at points where its cross-engine dependencies are already
settled, so the in-order PE queue never waits on ScalarE: score pairs
trail their tanh by 2 ob-groups, the chunk's last score pair + exp +
attn-transposes + ctx matmuls trail by one chunk.

The final out = tanh(Wout @ [ctx; q]) uses a hi/lo split-bf16 Wout and
split cat (3 accumulation chains), giving ~fp32 accuracy (~4e-3 rel).

softmax is computed without max-subtraction: scores stay O(1) for this
data; exp accumulates in fp32.

src_lengths is (faithfully to the reference) unused.
"""
import numpy as np
from contextlib import ExitStack

import concourse.bass as bass
import concourse.tile as tile
from concourse import bacc, mybir, masks
from concourse import bass_isa
from concourse import bass_utils

F32 = mybir.dt.float32
BF16 = mybir.dt.bfloat16
FP8 = mybir.dt.float8e4
Tanh = mybir.ActivationFunctionType.Tanh
Exp = mybir.ActivationFunctionType.Exp
Copy = mybir.ActivationFunctionType.Copy
DR = mybir.MatmulPerfMode.DoubleRow

B, T, S, H = 32, 1, 4096, 1024
NCORES = 8
BL = B // NCORES       # batches per core
NS = S // 128          # s-tiles per batch
CH = 4                 # s-tiles per enc chunk (512 positions)
NCH = NS // CH         # chunks per batch
NHB = H // 128         # h (contraction) blocks
NOB = H // 128         # o (output) blocks
NKB = 2 * H // 128     # k blocks of cat=[ctx;query]
PREFETCH = 3           # chunks of lookahead on the enc stream


def _build_program():
    nc = bacc.Bacc("TRN2", target_bir_lowering=False, debug=False)

    q_d = nc.dram_tensor("query", (BL, T, H), F32, kind="ExternalInput").ap()
    enc_d = nc.dram_tensor("encoder_outputs", (BL, S, H), F32,
                           kind="ExternalInput").ap()
    ws_d = nc.dram_tensor("Ws_w", (H, H), F32, kind="ExternalInput").ap()
    wh_d = nc.dram_tensor("Wh_w", (H, H), F32, kind="ExternalInput").ap()
    v_d = nc.dram_tensor("v_w", (1, H), F32, kind="ExternalInput").ap()
    wout_d = nc.dram_tensor("Wout_w", (H, 2 * H), F32, kind="ExternalInput").ap()
    out_d = nc.dram_tensor("out", (BL, T, H), F32, kind="ExternalOutput").ap()

    with tile.TileContext(nc) as tc, ExitStack() as ctx:
        # ---------------- pools ----------------
        wt_pool = ctx.enter_context(tc.tile_pool(name="wt", bufs=1))
        wstage_pool = ctx.enter_context(tc.tile_pool(name="wstage", bufs=1))
        wnat_pool = ctx.enter_context(tc.tile_pool(name="wnat", bufs=3))
        wof_pool = ctx.enter_context(tc.tile_pool(name="wof", bufs=1))
        wohl_pool = ctx.enter_context(tc.tile_pool(name="wohl", bufs=2))
        encN_pool = ctx.enter_context(tc.tile_pool(name="encN", bufs=5))
        encT_pool = ctx.enter_context(tc.tile_pool(name="encT", bufs=2))
        encT8_pool = ctx.enter_context(tc.tile_pool(name="encT8", bufs=4))
        t8_pool = ctx.enter_context(tc.tile_pool(name="t8", bufs=2))
        att_pool = ctx.enter_context(tc.tile_pool(name="att", bufs=2))
        small_pool = ctx.enter_context(tc.tile_pool(name="small", bufs=1))

        hp_psum = ctx.enter_context(tc.tile_pool(name="hp_ps", bufs=3, space="PSUM"))
        sc_psum = ctx.enter_context(tc.tile_pool(name="sc_ps", bufs=2, space="PSUM"))
        ctx_psum = ctx.enter_context(tc.tile_pool(name="ctx_ps", bufs=1, space="PSUM"))
        atp_psum = ctx.enter_context(tc.tile_pool(name="atp_ps", bufs=1, space="PSUM"))

        # ---------------- constants ----------------
        id1 = small_pool.tile([1, 1], F32)
        masks.make_identity(nc, id1[:])
        id4 = small_pool.tile([4, 4], F32)
        masks.make_identity(nc, id4[:])
        id8 = small_pool.tile([8, 8], F32)
        masks.make_identity(nc, id8[:])

        # ---------------- encoder chunk loader ----------------
        chunk_tiles = {}

        def chunk_dma(g):
            if g in chunk_tiles:
                return chunk_tiles[g]
            b, c = g // NCH, g % NCH
            encN = encN_pool.tile([128, CH, H], BF16, tag="encN")
            src = enc_d[b, c * CH * 128:(c + 1) * CH * 128, :]
            nc.gpsimd.dma_start(encN[:], src.rearrange("(t p) h -> p t h", p=128))
            encT = encT_pool.tile([128, CH * NHB, 128], BF16, tag="encT")
            nc.sync.dma_start(encT[:], encN[:], transpose=True)
            encT8 = encT8_pool.tile([128, CH, NHB, 128], FP8, tag="encT8")
            nc.vector.tensor_copy(
                encT8[:].rearrange("p t k i -> p (t k) i"), encT[:])
            chunk_tiles[g] = (encN, encT8)
            return chunk_tiles[g]

        # first enc chunks go out before anything else
        chunk_dma(0)
        chunk_dma(1)

        # ---------------- Wh path: nat casts -> PE transpose -> fp8 --------
        # (keeps the xbar/sync queue free for the enc stream at startup; the
        #  PE is idle here anyway and the transposes warm it up)
        id128 = small_pool.tile([128, 128], BF16)
        masks.make_identity(nc, id128[:])
        whT8 = wt_pool.tile([128, NHB, H], FP8, tag="whT8")
        wh_nat = []
        for j in range(NHB):
            wN = wnat_pool.tile([128, H], BF16, tag="wnat")
            nc.gpsimd.dma_start(wN[:], wh_d[j * 128:(j + 1) * 128, :])
            wh_nat.append(wN)
        chunk_dma(2)
        for j in range(NHB):
            for hq in range(2):
                tp = hp_psum.tile([128, 4, 128], BF16, tag="hp", name="tp")
                for hb in range(4):
                    nc.tensor.transpose(
                        tp[:, hb, :],
                        wh_nat[j][:, (hq * 4 + hb) * 128:(hq * 4 + hb + 1) * 128],
                        id128[:])
                nc.vector.tensor_copy(
                    whT8[:, hq * 4:(hq + 1) * 4, j * 128:(j + 1) * 128], tp[:])

        # ---------------- q / v / Ws path ----------------
        q_sb = small_pool.tile([BL, H], F32)
        nc.gpsimd.dma_start(q_sb[:], q_d[0:BL, 0, :])
        v_sb = small_pool.tile([NHB, 128], F32)
        for j in range(NHB):
            nc.gpsimd.dma_start(v_sb[j:j + 1, :], v_d[0:1, j * 128:(j + 1) * 128])

        wsT = wt_pool.tile([128, NHB, H], BF16, tag="wbig")
        ws_nat = []
        for j in range(NHB):
            wN = wnat_pool.tile([128, H], BF16, tag="wnat")
            nc.gpsimd.dma_start(wN[:], ws_d[j * 128:(j + 1) * 128, :])
            ws_nat.append(wN)
        for j in range(NHB):
            nc.sync.dma_start(wsT[:, :, j * 128:(j + 1) * 128], ws_nat[j][:],
                              transpose=True)

        # q transposed: [h, b] f32 -> bf16 for the q_proj matmuls, and the
        # hi/lo split halves of cat's query blocks
        cat_hi = small_pool.tile([128, NKB, BL], BF16)
        cat_lo = small_pool.tile([128, NKB, BL], BF16)
        qT = small_pool.tile([128, NHB, BL], BF16)
        qt_ps = atp_psum.tile([128, NHB * BL], F32, tag="atp")
        for j in range(NHB):
            nc.tensor.transpose(qt_ps[:, j * BL:(j + 1) * BL],
                                q_sb[0:BL, j * 128:(j + 1) * 128], id4[:])
        nc.scalar.copy(qT[:], qt_ps[:])
        nc.scalar.copy(cat_hi[:, NHB:NKB, :], qt_ps[:])
        nc.vector.tensor_sub(cat_lo[:, NHB:NKB, :], qt_ps[:],
                             cat_hi[:, NHB:NKB, :])

        # v transposed to [o%128, ob] and scaled x16 into fp8 (padded stride
        # 16 so DoubleRow pair-slices have a legal step)
        v8 = small_pool.tile([128, NHB, 16], FP8)
        vt_ps = atp_psum.tile([128, NHB], F32, tag="atp")
        nc.tensor.transpose(vt_ps[:], v_sb[:], id8[:])
        nc.scalar.activation(v8[:, :, 0:1], vt_ps[:].unsqueeze(-1), Copy,
                             scale=16.0)

        # q_proj^T[o, b] via PE, then to SBUF f32 as tanh's per-partition bias
        qpT = small_pool.tile([128, NOB, BL], F32)
        qp_ps = atp_psum.tile([128, NOB * BL], F32, tag="atp")
        for ob in range(NOB):
            for hb in range(NHB):
                nc.tensor.matmul(qp_ps[:, ob * BL:(ob + 1) * BL],
                                 wsT[:, hb, ob * 128:(ob + 1) * 128],
                                 qT[:, hb, :],
                                 start=(hb == 0), stop=(hb == NHB - 1))
        nc.scalar.copy(qpT[:], qp_ps[:].rearrange("p (o b) -> p o b", b=BL))

        # ---------------- Wout hi/lo prep (deferred, off critical path) ----
        woutT_hi = wt_pool.tile([128, NKB, H], BF16, tag="wbig")
        woutT_lo = wt_pool.tile([128, NKB, H], BF16, tag="wlo")
        for j in range(NHB):
            with tc.tile_wait_until(0.05 + j * 0.045):
                wof = wof_pool.tile([128, 2 * H], F32, tag="wof")
                nc.gpsimd.dma_start(wof[:], wout_d[j * 128:(j + 1) * 128, :])
                w_hi = wohl_pool.tile([128, 2 * H], BF16, tag="wohl")
                nc.gpsimd.dma_start(w_hi[:], wof[:])
                w_lo = wohl_pool.tile([128, 2 * H], BF16, tag="wohl")
                nc.vector.tensor_sub(w_lo[:], wof[:], w_hi[:])
            with tc.tile_wait_until(0.08 + j * 0.045):
                nc.sync.dma_start(woutT_hi[:, :, j * 128:(j + 1) * 128],
                                  w_hi[:], transpose=True)
                nc.sync.dma_start(woutT_lo[:, :, j * 128:(j + 1) * 128],
                                  w_lo[:], transpose=True)

        # ---------------- main loop (software-pipelined chunk tails) -------
        ctx_sb = small_pool.tile([BL, H], F32)
        NG = BL * NCH
        # per-chunk state carried into the next block
        st_t8 = {}
        st_sc = {}
        st_att = {}   # batch -> (attnT, dsum, ctx_ps)

        def hp_group(g, ob, encT8, t8, bi):
            hp = hp_psum.tile([128, 512], F32, tag="hp")
            enc_mv = encT8[:].rearrange("p t k i -> p k t i")
            for kp in range(NHB // 2):
                nc.tensor.matmul(
                    hp[:],
                    whT8[:, 2 * kp:2 * kp + 2, ob * 128:(ob + 1) * 128],
                    enc_mv[:, 2 * kp:2 * kp + 2, :, :],
                    start=(kp == 0), stop=(kp == NHB // 2 - 1),
                    perf_mode=DR)
            nc.scalar.activation(t8[:, ob, :], hp[:], Tanh,
                                 bias=qpT[:, ob, bi:bi + 1])

        def scores_mm(g, p, t8, sc_ps):
            nc.tensor.matmul(sc_ps[0:1, :],
                             v8[:, 2 * p:2 * p + 2, 0:1],
                             t8[:, 2 * p:2 * p + 2, :],
                             start=(p == 0), stop=(p == NOB // 2 - 1),
                             perf_mode=DR)

        def tail_a(g):
            # final score pair + exp for chunk g (tanh(g,6..7) long done)
            pb, pc = g // NCH, g % NCH
            scores_mm(g, NOB // 2 - 1, st_t8.pop(g), st_sc[g])
            attnT, dsum, _ = st_att[pb]
            attnU = att_pool.tile([1, 512], F32, tag="attnU")
            nc.scalar.activation(attnU[0:1, :], st_sc.pop(g)[0:1, :], Exp,
                                 scale=1.0 / 16.0,
                                 accum_out=dsum[0:1, pc:pc + 1])
            return attnU

        def tail_b(g, attnU):
            # attn row -> columns (PE) and bf16 copy (ACT)
            pb, pc = g // NCH, g % NCH
            attnT, _, _ = st_att[pb]
            atp = atp_psum.tile([128, CH], F32, tag="atp")
            for t in range(CH):
                nc.tensor.transpose(atp[:, t:t + 1],
                                    attnU[0:1, t * 128:(t + 1) * 128], id1[:])
            nc.scalar.copy(attnT[:, pc * CH:(pc + 1) * CH], atp[:])

        def tail_c(g):
            # ctx matmuls for chunk g
            pb, pc = g // NCH, g % NCH
            attnT, _, ctx_ps = st_att[pb]
            encN = chunk_tiles[g][0]
            for t in range(CH):
                st = pc * CH + t
                for half in range(2):
                    nc.tensor.matmul(
                        ctx_ps[0:1, half * 512:(half + 1) * 512],
                        attnT[:, st:st + 1],
                        encN[:, t, half * 512:(half + 1) * 512],
                        start=(st == 0), stop=(st == NS - 1))

        def tail_d(g):
            # batch finish: denominator, 1/d, normalized ctx row
            pb = g // NCH
            _, dsum, ctx_ps = st_att.pop(pb)
            dtot = att_pool.tile([1, 1], F32, tag="dtot")
            nc.vector.reduce_sum(dtot[:], dsum[:], axis=mybir.AxisListType.X)
            inv_d = att_pool.tile([1, 1], F32, tag="invd")
            nc.vector.reciprocal(inv_d[:], dtot[:])
            ctx_row = wstage_pool.tile([1, H], F32, tag="ctxrow")
            nc.scalar.activation(ctx_row[:], ctx_ps[:], Copy,
                                 scale=inv_d[0:1, 0:1])
            nc.scalar.dma_start(ctx_sb[pb:pb + 1, :], ctx_row[:])

        for g in range(NG):
            b, c = g // NCH, g % NCH
            if c == 0:
                st_att[b] = (
                    att_pool.tile([128, NS], BF16, tag="attnT", name="attnT"),
                    att_pool.tile([1, NCH], F32, tag="dsum", name="dsum"),
                    ctx_psum.tile([1, H], F32, tag="ctx", name="ctx_ps"))
            encN, encT8 = chunk_dma(g)
            if g + PREFETCH < NG:
                chunk_dma(g + PREFETCH)
            t8 = t8_pool.tile([128, NOB, 512], FP8, tag="t8")
            sc_ps = sc_psum.tile([1, 512], F32, tag="sc")

            hp_group(g, 0, encT8, t8, b)
            if g >= 1:
                attnU_prev = tail_a(g - 1)
            hp_group(g, 1, encT8, t8, b)
            hp_group(g, 2, encT8, t8, b)
            if g >= 1:
                tail_b(g - 1, attnU_prev)
            hp_group(g, 3, encT8, t8, b)
            scores_mm(g, 0, t8, sc_ps)
            if g >= 1:
                tail_c(g - 1)
                if (g - 1) % NCH == NCH - 1:
                    tail_d(g - 1)
            hp_group(g, 4, encT8, t8, b)
            hp_group(g, 5, encT8, t8, b)
            scores_mm(g, 1, t8, sc_ps)
            hp_group(g, 6, encT8, t8, b)
            hp_group(g, 7, encT8, t8, b)
            scores_mm(g, 2, t8, sc_ps)
            st_t8[g] = t8
            st_sc[g] = sc_ps

        # epilogue for the last chunk
        attnU_prev = tail_a(NG - 1)
        tail_b(NG - 1, attnU_prev)
        tail_c(NG - 1)
        tail_d(NG - 1)

        # ---------------- finale ----------------
        ct_ps = atp_psum.tile([128, NHB * BL], F32, tag="atp")
        for j in range(NHB):
            nc.tensor.transpose(ct_ps[:, j * BL:(j + 1) * BL],
                                ctx_sb[0:BL, j * 128:(j + 1) * 128], id4[:])
        nc.scalar.copy(cat_hi[:, 0:NHB, :], ct_ps[:])
        nc.vector.tensor_sub(cat_lo[:, 0:NHB, :], ct_ps[:], cat_hi[:, 0:NHB, :])

        out_sb = wstage_pool.tile([BL, H], F32, tag="ctxrow")
        chains = [(cat_hi, woutT_hi), (cat_hi, woutT_lo), (cat_lo, woutT_hi)]
        for half in range(2):
            out_ps = sc_psum.tile([BL, 512], F32, tag="sc")
            n = 0
            for ca, wb in chains:
                for kb in range(NKB):
                    nc.tensor.matmul(out_ps[:], ca[:, kb, :],
                                     wb[:, kb, half * 512:(half + 1) * 512],
                                     start=(n == 0),
                                     stop=(n == 3 * NKB - 1))
                    n += 1
            nc.scalar.activation(out_sb[:, half * 512:(half + 1) * 512],
                                 out_ps[:], Tanh)
        nc.scalar.dma_start(out_d[0:BL, 0, :], out_sb[:])

    nc.compile()
    return nc


_program = None


def get_program():
    global _program
    if _program is None:
        _program = _build_program()
    return _program


def run_sharded(inputs, trace=False, **kw):
    nc = get_program()
    in_maps = []
    for i in range(NCORES):
        sl = slice(i * BL, (i + 1) * BL)
        in_maps.append({
            "query": np.ascontiguousarray(inputs["query"][sl], dtype=np.float32),
            "encoder_outputs": np.ascontiguousarray(
                inputs["encoder_outputs"][sl], dtype=np.float32),
            "Ws_w": np.asarray(inputs["Ws_w"], dtype=np.float32),
            "Wh_w": np.asarray(inputs["Wh_w"], dtype=np.float32),
            "v_w": np.asarray(inputs["v_w"], dtype=np.float32),
            "Wout_w": np.asarray(inputs["Wout_w"], dtype=np.float32),
        })
    res = bass_utils.run_bass_kernel_spmd(
        nc, in_maps, core_ids=list(range(NCORES)), trace=trace, **kw)
    out = np.concatenate(
        [np.asarray(res.results[i]["out"], dtype=np.float32).reshape(BL, T, H)
         for i in range(NCORES)], axis=0)
    return out, res


def kernel(**inputs):
    out, _ = run_sharded(inputs)
    return out


# revision 15
# speedup vs baseline: 1.1858x; 1.0295x over previous
"""Bahdanau attention Trainium2 kernel (transposed-hp design).

B=32, T=1, S=4096, H=1024. Data-parallel over batch across 8 NeuronCores
(4 batches/core). Per core, a single-pass streaming kernel built around a
transposed h_proj layout hp^T[o, s] so that:

  - the q_proj bias-add fuses into ScalarE's tanh as a per-partition bias
    (VectorE drops out of the inner loop entirely)
  - the v-dot score reduction becomes cheap DoubleRow fp8 matmuls
    (v as a [K,1] stationary)
  - TensorE runs dense (h_proj fp8-DR, score MMs, ctx MMs, final MMs)
    and stays HAM-warm

Per chunk of 512 encoder positions: enc streams HBM->bf16 natural (SWDGE
cast) -> xbar transpose (bf16) -> fp8 copy on VectorE. h_proj^T[o,s]
accumulates Wh^T(fp8,stationary) x encT8(fp8,moving) with DoubleRow;
ScalarE applies tanh(hp + q_proj[o]) writing fp8 tiles; v^T-dot scores via
fp8-DR MMs into a [1,512] PSUM row; exp (scale=1/16 compensates v8's x16)
accumulates the softmax denominator; tiny PE transposes give attn columns
for the bf16 ctx matmuls.

Each chunk's softmax/ctx tail is software-pipelined into the NEXT chunk's
emission at points where its cross-engine dependencies are already
settled, so the in-order PE queue never waits on ScalarE: score pairs
trail their tanh by 2 ob-groups, the chunk's last score pair + exp +
attn-transposes + ctx matmuls trail by one chunk.

The final out = tanh(Wout @ [ctx; q]) uses a hi/lo split-bf16 Wout and
split cat (3 accumulation chains), giving ~fp32 accuracy (~4e-3 rel).

softmax is computed without max-subtraction: scores stay O(1) for this
data; exp accumulates in fp32.

src_lengths is (faithfully to the reference) unused.
"""
import numpy as np
from contextlib import ExitStack

import concourse.bass as bass
import concourse.tile as tile
from concourse import bacc, mybir, masks
from concourse import bass_isa
from concourse import bass_utils

F32 = mybir.dt.float32
BF16 = mybir.dt.bfloat16
FP8 = mybir.dt.float8e4
Tanh = mybir.ActivationFunctionType.Tanh
Exp = mybir.ActivationFunctionType.Exp
Copy = mybir.ActivationFunctionType.Copy
DR = mybir.MatmulPerfMode.DoubleRow

B, T, S, H = 32, 1, 4096, 1024
NCORES = 8
BL = B // NCORES       # batches per core
NS = S // 128          # s-tiles per batch
CH = 4                 # s-tiles per enc chunk (512 positions)
NCH = NS // CH         # chunks per batch
NHB = H // 128         # h (contraction) blocks
NOB = H // 128         # o (output) blocks
NKB = 2 * H // 128     # k blocks of cat=[ctx;query]
PREFETCH = 3           # chunks of lookahead on the enc stream


def _build_program():
    nc = bacc.Bacc("TRN2", target_bir_lowering=False, debug=False)

    q_d = nc.dram_tensor("query", (BL, T, H), F32, kind="ExternalInput").ap()
    enc_d = nc.dram_tensor("encoder_outputs", (BL, S, H), F32,
                           kind="ExternalInput").ap()
    ws_d = nc.dram_tensor("Ws_w", (H, H), F32, kind="ExternalInput").ap()
    wh_d = nc.dram_tensor("Wh_w", (H, H), F32, kind="ExternalInput").ap()
    v_d = nc.dram_tensor("v_w", (1, H), F32, kind="ExternalInput").ap()
    wout_d = nc.dram_tensor("Wout_w", (H, 2 * H), F32, kind="ExternalInput").ap()
    out_d = nc.dram_tensor("out", (BL, T, H), F32, kind="ExternalOutput").ap()

    with tile.TileContext(nc) as tc, ExitStack() as ctx:
        # ---------------- pools ----------------
        wt_pool = ctx.enter_context(tc.tile_pool(name="wt", bufs=1))
        wstage_pool = ctx.enter_context(tc.tile_pool(name="wstage", bufs=1))
        wnat_pool = ctx.enter_context(tc.tile_pool(name="wnat", bufs=3))
        wof_pool = ctx.enter_context(tc.tile_pool(name="wof", bufs=1))
        wohl_pool = ctx.enter_context(tc.tile_pool(name="wohl", bufs=2))
        encN_pool = ctx.enter_context(tc.tile_pool(name="encN", bufs=6))
        encT_pool = ctx.enter_context(tc.tile_pool(name="encT", bufs=3))
        encT8_pool = ctx.enter_context(tc.tile_pool(name="encT8", bufs=4))
        t8_pool = ctx.enter_context(tc.tile_pool(name="t8", bufs=2))
        att_pool = ctx.enter_context(tc.tile_pool(name="att", bufs=2))
        small_pool = ctx.enter_context(tc.tile_pool(name="small", bufs=1))

        hp_psum = ctx.enter_context(tc.tile_pool(name="hp_ps", bufs=3, space="PSUM"))
        sc_psum = ctx.enter_context(tc.tile_pool(name="sc_ps", bufs=2, space="PSUM"))
        ctx_psum = ctx.enter_context(tc.tile_pool(name="ctx_ps", bufs=1, space="PSUM"))
        atp_psum = ctx.enter_context(tc.tile_pool(name="atp_ps", bufs=1, space="PSUM"))

        # ---------------- constants ----------------
        id1 = small_pool.tile([1, 1], F32)
        masks.make_identity(nc, id1[:])
        id4 = small_pool.tile([4, 4], F32)
        masks.make_identity(nc, id4[:])
        id8 = small_pool.tile([8, 8], F32)
        masks.make_identity(nc, id8[:])

        # ---------------- encoder chunk loader ----------------
        chunk_tiles = {}

        def chunk_dma(g):
            if g in chunk_tiles:
                return chunk_tiles[g]
            b, c = g // NCH, g % NCH
            encN = encN_pool.tile([128, CH, H], BF16, tag="encN")
            src = enc_d[b, c * CH * 128:(c + 1) * CH * 128, :]
            nc.gpsimd.dma_start(encN[:], src.rearrange("(t p) h -> p t h", p=128))
            encT = encT_pool.tile([128, CH * NHB, 128], BF16, tag="encT")
            nc.sync.dma_start(encT[:], encN[:], transpose=True)
            encT8 = encT8_pool.tile([128, CH, NHB, 128], FP8, tag="encT8")
            nc.vector.tensor_copy(
                encT8[:].rearrange("p t k i -> p (t k) i"), encT[:])
            chunk_tiles[g] = (encN, encT8)
            return chunk_tiles[g]

        # first enc chunks go out before anything else
        chunk_dma(0)
        chunk_dma(1)

        # ---------------- Wh path: nat casts -> PE transpose -> fp8 --------
        # (keeps the xbar/sync queue free for the enc stream at startup; the
        #  PE is idle here anyway and the transposes warm it up)
        id128 = small_pool.tile([128, 128], BF16)
        masks.make_identity(nc, id128[:])
        whT8 = wt_pool.tile([128, NHB, H], FP8, tag="whT8")
        wh_nat = []
        for j in range(NHB):
            wN = wnat_pool.tile([128, H], BF16, tag="wnat")
            nc.gpsimd.dma_start(wN[:], wh_d[j * 128:(j + 1) * 128, :])
            wh_nat.append(wN)
        chunk_dma(2)
        for j in range(NHB):
            for hq in range(2):
                tp = hp_psum.tile([128, 4, 128], BF16, tag="hp", name="tp")
                for hb in range(4):
                    nc.tensor.transpose(
                        tp[:, hb, :],
                        wh_nat[j][:, (hq * 4 + hb) * 128:(hq * 4 + hb + 1) * 128],
                        id128[:])
                nc.vector.tensor_copy(
                    whT8[:, hq * 4:(hq + 1) * 4, j * 128:(j + 1) * 128], tp[:])

        # ---------------- q / v / Ws path ----------------
        q_sb = small_pool.tile([BL, H], F32)
        nc.gpsimd.dma_start(q_sb[:], q_d[0:BL, 0, :])
        v_sb = small_pool.tile([NHB, 128], F32)
        for j in range(NHB):
            nc.gpsimd.dma_start(v_sb[j:j + 1, :], v_d[0:1, j * 128:(j + 1) * 128])

        wsT = wt_pool.tile([128, NHB, H], BF16, tag="wbig")
        ws_nat = []
        for j in range(NHB):
            wN = wnat_pool.tile([128, H], BF16, tag="wnat")
            nc.gpsimd.dma_start(wN[:], ws_d[j * 128:(j + 1) * 128, :])
            ws_nat.append(wN)
        for j in range(NHB):
            nc.sync.dma_start(wsT[:, :, j * 128:(j + 1) * 128], ws_nat[j][:],
                              transpose=True)

        # q transposed: [h, b] f32 -> bf16 for the q_proj matmuls, and the
        # hi/lo split halves of cat's query blocks
        cat_hi = small_pool.tile([128, NKB, BL], BF16)
        cat_lo = small_pool.tile([128, NKB, BL], BF16)
        qT = small_pool.tile([128, NHB, BL], BF16)
        qt_ps = atp_psum.tile([128, NHB * BL], F32, tag="atp")
        for j in range(NHB):
            nc.tensor.transpose(qt_ps[:, j * BL:(j + 1) * BL],
                                q_sb[0:BL, j * 128:(j + 1) * 128], id4[:])
        nc.scalar.copy(qT[:], qt_ps[:])
        nc.scalar.copy(cat_hi[:, NHB:NKB, :], qt_ps[:])
        nc.vector.tensor_sub(cat_lo[:, NHB:NKB, :], qt_ps[:],
                             cat_hi[:, NHB:NKB, :])

        # v transposed to [o%128, ob] and scaled x16 into fp8 (padded stride
        # 16 so DoubleRow pair-slices have a legal step)
        v8 = small_pool.tile([128, NHB, 16], FP8)
        vt_ps = atp_psum.tile([128, NHB], F32, tag="atp")
        nc.tensor.transpose(vt_ps[:], v_sb[:], id8[:])
        nc.scalar.activation(v8[:, :, 0:1], vt_ps[:].unsqueeze(-1), Copy,
                             scale=16.0)

        # q_proj^T[o, b] via PE, then to SBUF f32 as tanh's per-partition bias
        qpT = small_pool.tile([128, NOB, BL], F32)
        qp_ps = atp_psum.tile([128, NOB * BL], F32, tag="atp")
        for ob in range(NOB):
            for hb in range(NHB):
                nc.tensor.matmul(qp_ps[:, ob * BL:(ob + 1) * BL],
                                 wsT[:, hb, ob * 128:(ob + 1) * 128],
                                 qT[:, hb, :],
                                 start=(hb == 0), stop=(hb == NHB - 1))
        nc.scalar.copy(qpT[:], qp_ps[:].rearrange("p (o b) -> p o b", b=BL))

        # ---------------- Wout hi/lo prep (deferred, off critical path) ----
        woutT_hi = wt_pool.tile([128, NKB, H], BF16, tag="wbig")
        woutT_lo = wt_pool.tile([128, NKB, H], BF16, tag="wlo")
        for j in range(NHB):
            with tc.tile_wait_until(0.05 + j * 0.045):
                wof = wof_pool.tile([128, 2 * H], F32, tag="wof")
                nc.gpsimd.dma_start(wof[:], wout_d[j * 128:(j + 1) * 128, :])
                w_hi = wohl_pool.tile([128, 2 * H], BF16, tag="wohl")
                nc.gpsimd.dma_start(w_hi[:], wof[:])
                w_lo = wohl_pool.tile([128, 2 * H], BF16, tag="wohl")
                nc.vector.tensor_sub(w_lo[:], wof[:], w_hi[:])
            with tc.tile_wait_until(0.08 + j * 0.045):
                nc.sync.dma_start(woutT_hi[:, :, j * 128:(j + 1) * 128],
                                  w_hi[:], transpose=True)
                nc.sync.dma_start(woutT_lo[:, :, j * 128:(j + 1) * 128],
                                  w_lo[:], transpose=True)

        # ---------------- main loop (software-pipelined chunk tails) -------
        ctx_sb = small_pool.tile([BL, H], F32)
        NG = BL * NCH
        # per-chunk state carried into the next block
        st_t8 = {}
        st_sc = {}
        st_att = {}   # batch -> (attnT, dsum, ctx_ps)

        def hp_group(g, ob, encT8, t8, bi):
            hp = hp_psum.tile([128, 512], F32, tag="hp")
            enc_mv = encT8[:].rearrange("p t k i -> p k t i")
            for kp in range(NHB // 2):
                nc.tensor.matmul(
                    hp[:],
                    whT8[:, 2 * kp:2 * kp + 2, ob * 128:(ob + 1) * 128],
                    enc_mv[:, 2 * kp:2 * kp + 2, :, :],
                    start=(kp == 0), stop=(kp == NHB // 2 - 1),
                    perf_mode=DR)
            nc.scalar.activation(t8[:, ob, :], hp[:], Tanh,
                                 bias=qpT[:, ob, bi:bi + 1])

        def scores_mm(g, p, t8, sc_ps):
            nc.tensor.matmul(sc_ps[0:1, :],
                             v8[:, 2 * p:2 * p + 2, 0:1],
                             t8[:, 2 * p:2 * p + 2, :],
                             start=(p == 0), stop=(p == NOB // 2 - 1),
                             perf_mode=DR)

        def tail_a(g):
            # final score pair + exp for chunk g (tanh(g,6..7) long done)
            pb, pc = g // NCH, g % NCH
            scores_mm(g, NOB // 2 - 1, st_t8.pop(g), st_sc[g])
            attnT, dsum, _ = st_att[pb]
            attnU = att_pool.tile([1, 512], F32, tag="attnU")
            nc.scalar.activation(attnU[0:1, :], st_sc.pop(g)[0:1, :], Exp,
                                 scale=1.0 / 16.0,
                                 accum_out=dsum[0:1, pc:pc + 1])
            return attnU

        def tail_b(g, attnU):
            # attn row -> columns (PE) and bf16 copy (ACT)
            pb, pc = g // NCH, g % NCH
            attnT, _, _ = st_att[pb]
            atp = atp_psum.tile([128, CH], F32, tag="atp")
            for t in range(CH):
                nc.tensor.transpose(atp[:, t:t + 1],
                                    attnU[0:1, t * 128:(t + 1) * 128], id1[:])
            nc.scalar.copy(attnT[:, pc * CH:(pc + 1) * CH], atp[:])

        def tail_c(g):
            # ctx matmuls for chunk g
            pb, pc = g // NCH, g % NCH
            attnT, _, ctx_ps = st_att[pb]
            encN = chunk_tiles[g][0]
            for t in range(CH):
                st = pc * CH + t
                for half in range(2):
                    nc.tensor.matmul(
                        ctx_ps[0:1, half * 512:(half + 1) * 512],
                        attnT[:, st:st + 1],
                        encN[:, t, half * 512:(half + 1) * 512],
                        start=(st == 0), stop=(st == NS - 1))

        def tail_d(g):
            # batch finish: denominator, 1/d, normalized ctx row
            pb = g // NCH
            _, dsum, ctx_ps = st_att.pop(pb)
            dtot = att_pool.tile([1, 1], F32, tag="dtot")
            nc.vector.reduce_sum(dtot[:], dsum[:], axis=mybir.AxisListType.X)
            inv_d = att_pool.tile([1, 1], F32, tag="invd")
            nc.vector.reciprocal(inv_d[:], dtot[:])
            ctx_row = wstage_pool.tile([1, H], F32, tag="ctxrow")
            nc.scalar.activation(ctx_row[:], ctx_ps[:], Copy,
                                 scale=inv_d[0:1, 0:1])
            nc.scalar.dma_start(ctx_sb[pb:pb + 1, :], ctx_row[:])

        for g in range(NG):
            b, c = g // NCH, g % NCH
            if c == 0:
                st_att[b] = (
                    att_pool.tile([128, NS], BF16, tag="attnT", name="attnT"),
                    att_pool.tile([1, NCH], F32, tag="dsum", name="dsum"),
                    ctx_psum.tile([1, H], F32, tag="ctx", name="ctx_ps"))
            encN, encT8 = chunk_dma(g)
            if g + PREFETCH < NG:
                chunk_dma(g + PREFETCH)
            t8 = t8_pool.tile([128, NOB, 512], FP8, tag="t8")
            sc_ps = sc_psum.tile([1, 512], F32, tag="sc")

            hp_group(g, 0, encT8, t8, b)
            if g >= 1:
                attnU_prev = tail_a(g - 1)
            hp_group(g, 1, encT8, t8, b)
            hp_group(g, 2, encT8, t8, b)
            if g >= 1:
                tail_b(g - 1, attnU_prev)
            hp_group(g, 3, encT8, t8, b)
            scores_mm(g, 0, t8, sc_ps)
            if g >= 1:
                tail_c(g - 1)
                if (g - 1) % NCH == NCH - 1:
                    tail_d(g - 1)
            hp_group(g, 4, encT8, t8, b)
            hp_group(g, 5, encT8, t8, b)
            scores_mm(g, 1, t8, sc_ps)
            hp_group(g, 6, encT8, t8, b)
            hp_group(g, 7, encT8, t8, b)
            scores_mm(g, 2, t8, sc_ps)
            st_t8[g] = t8
            st_sc[g] = sc_ps

        # epilogue for the last chunk
        attnU_prev = tail_a(NG - 1)
        tail_b(NG - 1, attnU_prev)
        tail_c(NG - 1)
        tail_d(NG - 1)

        # ---------------- finale ----------------
        ct_ps = atp_psum.tile([128, NHB * BL], F32, tag="atp")
        for j in range(NHB):
            nc.tensor.transpose(ct_ps[:, j * BL:(j + 1) * BL],
                                ctx_sb[0:BL, j * 128:(j + 1) * 128], id4[:])
        nc.scalar.copy(cat_hi[:, 0:NHB, :], ct_ps[:])
        nc.vector.tensor_sub(cat_lo[:, 0:NHB, :], ct_ps[:], cat_hi[:, 0:NHB, :])

        out_sb = wstage_pool.tile([BL, H], F32, tag="ctxrow")
        chains = [(cat_hi, woutT_hi), (cat_hi, woutT_lo), (cat_lo, woutT_hi)]
        for half in range(2):
            out_ps = sc_psum.tile([BL, 512], F32, tag="sc")
            n = 0
            for ca, wb in chains:
                for kb in range(NKB):
                    nc.tensor.matmul(out_ps[:], ca[:, kb, :],
                                     wb[:, kb, half * 512:(half + 1) * 512],
                                     start=(n == 0),
                                     stop=(n == 3 * NKB - 1))
                    n += 1
            nc.scalar.activation(out_sb[:, half * 512:(half + 1) * 512],
                                 out_ps[:], Tanh)
        nc.scalar.dma_start(out_d[0:BL, 0, :], out_sb[:])

    nc.compile()
    return nc


_program = None


def get_program():
    global _program
    if _program is None:
        _program = _build_program()
    return _program


def run_sharded(inputs, trace=False, **kw):
    nc = get_program()
    in_maps = []
    for i in range(NCORES):
        sl = slice(i * BL, (i + 1) * BL)
        in_maps.append({
            "query": np.ascontiguousarray(inputs["query"][sl], dtype=np.float32),
            "encoder_outputs": np.ascontiguousarray(
                inputs["encoder_outputs"][sl], dtype=np.float32),
            "Ws_w": np.asarray(inputs["Ws_w"], dtype=np.float32),
            "Wh_w": np.asarray(inputs["Wh_w"], dtype=np.float32),
            "v_w": np.asarray(inputs["v_w"], dtype=np.float32),
            "Wout_w": np.asarray(inputs["Wout_w"], dtype=np.float32),
        })
    res = bass_utils.run_bass_kernel_spmd(
        nc, in_maps, core_ids=list(range(NCORES)), trace=trace, **kw)
    out = np.concatenate(
        [np.asarray(res.results[i]["out"], dtype=np.float32).reshape(BL, T, H)
         for i in range(NCORES)], axis=0)
    return out, res


def kernel(**inputs):
    out, _ = run_sharded(inputs)
    return out
